# revision 43
# baseline (speedup 1.0000x reference)
"""Cross-attention-concat kernel for Trainium2 (8 NeuronCores, Bass/Tile).

Math (per batch b):
  x   = concat(rgb, chm) on channels           [512, 4096]   (pixels hw = h*64+w)
  Q   = Wq x + bq ; K = Wk x + bk              [64, ...]
  V   = Wv x + bv                              [256, 4096]
  S   = Q^T K                                  [2048 hw, 4096 xy]
  A   = softmax over y within each x-group of 64 keys
  out = Wcr (A V^T)^T + bcr                    [256, 2048]

Sharding: core = (batch, H-half). The host rolls each batch's pixel axis by
2048*(core%2) so every core runs the same program with its queries at
columns 0:2048 of the rolled image.

Key structure (v2):
- Transposed-scores formulation: S^T [xy, q] per 128-xy block, exp on
  ScalarE, per-x-group softmax denominators collected by a selector matmul
  (g64), reciprocal broadcast back by a second selector matmul (g128),
  scale fused into the PSUM->SBUF pass feeding A@V.
- The two K=64-contraction matmuls (scores: c=64; 1/d broadcast: rd rows)
  are ROW-PACKED: two concurrent matmuls on PE row-halves via
  tile_position, halving their stream time. K blocks are packed
  even/odd into partition halves (k2), Q is duplicated into both halves,
  rd carries d duplicated into both halves (via g64's dup columns).
- Preamble is DMA-pipelined column-chunk-wise with a fused [Wq|Wk]
  stationary; Q/K land via small SBUF->SBUF shuffle DMAs.
- Reciprocal uses the fast approx DVE op and is emitted one pipeline
  stage early so it never stalls the PE.
"""

import numpy as np
import ml_dtypes

B, C, H, W = 4, 256, 64, 64
HW = H * W               # 4096
CIN = 2 * C              # 512
QCOLS = HW // 2          # 2048 queries per core
NSUP = QCOLS // 512      # 4 super-blocks of 512 queries
NI = HW // 128           # 32 xy-blocks of 128
NP = NI // 2             # 16 block-pairs

_CACHE = {}


def _build_nc():
    import concourse.bacc as bacc
    import concourse.tile as tile
    from concourse import mybir

    F32 = mybir.dt.float32
    BF16 = mybir.dt.bfloat16
    AF = mybir.ActivationFunctionType

    nc = bacc.Bacc("TRN2", target_bir_lowering=False, debug=False, num_devices=8)

    x_d = nc.dram_tensor("x", [CIN, HW], BF16, kind="ExternalInput").ap()
    g64_d = nc.dram_tensor("g64b", [128, 190], BF16, kind="ExternalInput").ap()
    g128_d = nc.dram_tensor("g128p", [128, 16, 128], BF16, kind="ExternalInput").ap()
    wqk_d = nc.dram_tensor("wqk", [128, 4, 128], BF16, kind="ExternalInput").ap()
    # wvt carries (Wcr @ Wv)^T: the output projection is folded into V on the
    # host, so the A@V matmul directly produces the projected output
    wvt_d = nc.dram_tensor("wvt", [128, 4, 256], BF16, kind="ExternalInput").ap()
    bqk_d = nc.dram_tensor("bqk", [128, 1], F32, kind="ExternalInput").ap()
    bfin_d = nc.dram_tensor("bfin", [128, 2], F32, kind="ExternalInput").ap()
    out_d = nc.dram_tensor("out", [C, QCOLS], BF16, kind="ExternalOutput").ap()

    with tile.TileContext(nc) as tc:
        with (
            tc.tile_pool(name="const", bufs=1) as constp,
            tc.tile_pool(name="qkv", bufs=1) as qkvp,
            tc.tile_pool(name="pbuf", bufs=40) as pbufp,
            tc.tile_pool(name="esbuf", bufs=2) as esp,
            tc.tile_pool(name="rdbuf", bufs=2) as rdp,
            tc.tile_pool(name="obuf", bufs=2) as obufp,
        ):
            # ---- constants (weights first; big selector consts after x) ----
            wqk_sb = constp.tile([128, 4, 128], BF16)
            bqk_sb = constp.tile([128, 1], F32)
            nc.sync.dma_start(out=wqk_sb, in_=wqk_d)
            nc.sync.dma_start(out=bqk_sb, in_=bqk_d)

            q_sb = qkvp.tile([128, QCOLS], BF16)       # Q dup'd in both halves
            k2_sb = qkvp.tile([128, NP, 128], BF16)    # K blocks even/odd packed
            vt_sb = qkvp.tile([128, NI, 256], BF16)    # V^T [xy-block, 128, 256]

            wvt_sb = constp.tile([128, 4, 256], BF16)
            bfin_sb = constp.tile([128, 2], F32)
            g64_sb = constp.tile([128, 190], BF16)
            g128_sb = constp.tile([128, 16, 128], BF16)

            # ---- preamble: 2 big x chunks, warm-up MMs during the DMA head,
            # QK projection only (V is interleaved into S0 below) ----
            x_sb = [qkvp.tile([128, HW], BF16, tag=f"x{k}", name=f"x{k}") for k in range(4)]
            # qkb[:, j, a, b, :] : col 512j+256a+128b; rows 0:64 = Q, 64:128 = K
            qkb = qkvp.tile([128, 8, 2, 2, 128], BF16, name="qkb")

            def emit_v2(p, pool, tag, bufs=1):
                # V^T for blocks (2p, 2p+1)
                psv = pool.tile([128, 512], F32, tag=tag, bufs=bufs, name="psv")
                for bb in range(2):
                    i = 2 * p + bb
                    for k in range(4):
                        nc.tensor.matmul(
                            psv[:, bb * 256:(bb + 1) * 256],
                            lhsT=x_sb[k][:, i * 128:(i + 1) * 128],
                            rhs=wvt_sb[:, k, :],
                            start=(k == 0),
                            stop=(k == 3),
                        )
                nc.vector.tensor_copy(vt_sb[:, 2 * p:2 * p + 2, :], psv)

            def emit_qk(j, pool, tag, bufs=1):
                sl = slice(j * 512, (j + 1) * 512)
                psqk = pool.tile([128, 512], F32, tag=tag, bufs=bufs, name="psqk")
                for k in range(4):
                    nc.tensor.matmul(
                        psqk,
                        lhsT=wqk_sb[:, k, :],
                        rhs=x_sb[k][:, sl],
                        start=(k == 0),
                        stop=(k == 3),
                    )
                if j % 2 == 0:
                    nc.scalar.add(qkb[:, j], psqk, bqk_sb)
                else:
                    nc.vector.tensor_scalar_add(qkb[:, j], psqk, bqk_sb)
            with tc.tile_pool(name="ps_pre", bufs=1, space="PSUM") as ps_pre:
                # warm-up MMs: trip the PE HAM throttle to full clock and keep
                # it busy while the x chunks stream in. The operand comes from
                # an on-device iota so warm-up starts before any DMA lands.
                warm_sb = qkvp.tile([128, 512], BF16, name="warm_sb")
                nc.gpsimd.iota(warm_sb, [[1, 512]], base=0, channel_multiplier=0,
                               allow_small_or_imprecise_dtypes=True)
                junk = ps_pre.tile([128, 512], F32, tag="junk", name="junk")
                for _ in range(26):
                    nc.tensor.matmul(junk, lhsT=warm_sb[:, 0:128], rhs=warm_sb, start=True, stop=True)
                # x descriptors split across the two HWDGE engines (Sync +
                # Scalar) so descriptor generation isn't serialized; first
                # chunk small so the QK matmuls can start early. wvt leads the
                # scalar queue — the preamble V pairs need it right away.
                nc.scalar.dma_start(out=wvt_sb, in_=wvt_d)
                for lo, hi in ((0, 1024), (1024, 2048), (2048, 4096)):
                    for k in range(4):
                        eng = nc.sync if k < 2 else nc.scalar
                        eng.dma_start(out=x_sb[k][:, lo:hi], in_=x_d[k * 128:(k + 1) * 128, lo:hi])
                nc.sync.dma_start(out=g64_sb, in_=g64_d)
                nc.scalar.dma_start(out=g128_sb, in_=g128_d)
                nc.sync.dma_start(out=bfin_sb, in_=bfin_d)

                # interleave early V pairs as PE filler between QK chunks —
                # they only need x columns that have already landed, covering
                # the DMA wait for the next x chunk
                emit_qk(0, ps_pre, "psqk", 2)
                emit_qk(1, ps_pre, "psqk", 2)
                for p in range(4):
                    emit_v2(p, ps_pre, "psv", 2)
                emit_qk(2, ps_pre, "psqk", 2)
                emit_qk(3, ps_pre, "psqk", 2)
                for p in range(4, 8):
                    emit_v2(p, ps_pre, "psv", 2)
                # Q dup: both partition halves get the same biased Q
                nc.sync.dma_start(out=q_sb[0:64, :], in_=qkb[0:64, 0:4])
                nc.sync.dma_start(out=q_sb[64:128, :], in_=qkb[0:64, 0:4])
                # K shuffle halves: even xy-blocks -> window0, odd -> window1
                nc.sync.dma_start(out=k2_sb[0:64, 0:8, :], in_=qkb[64:128, 0:4, :, 0, :])
                nc.sync.dma_start(out=k2_sb[64:128, 0:8, :], in_=qkb[64:128, 0:4, :, 1, :])

            # ---- main loop ----
            with (
                tc.tile_pool(name="ps_sc", bufs=1, space="PSUM") as ps_sc,
                tc.tile_pool(name="ps_d", bufs=1, space="PSUM") as ps_d,
                tc.tile_pool(name="ps_bc", bufs=1, space="PSUM") as ps_bc,
                tc.tile_pool(name="ps_att", bufs=1, space="PSUM") as ps_att,
            ):
                e_tiles = {}   # (S, p) -> exp pair tile, bf16 [128, 2, 512]
                d_ps = {}      # S -> PSUM [128, 512] per-x-group sums (dup'd halves)
                rd_sb = {}     # S -> SBUF bf16 [128, 512] reciprocal sums
                att_h = {}     # S -> accumulating PSUM pair

                def emit_scores(S, p):
                    # pair p -> xy-blocks (2p, 2p+1) on PE row-halves.
                    # Separate single-buffered banks + per-block exps keep the
                    # PE->ScalarE->PE reuse chain shorter than the PE work per
                    # iteration, so the PE never waits.
                    q0 = S * 512
                    for w in range(2):
                        ps = ps_sc.tile([128, 512], F32, tag=f"sc{w}", name=f"sc{w}")
                        nc.tensor.matmul(
                            ps,
                            lhsT=k2_sb[64 * w:64 * w + 64, p, :],
                            rhs=q_sb[64 * w:64 * w + 64, q0:q0 + 512],
                            start=True,
                            stop=True,
                        )
                        e = pbufp.tile([128, 512], BF16, tag="e", name="e")
                        nc.scalar.activation(e, ps, AF.Exp)
                        e_tiles[(S, 2 * p + w)] = e

                def emit_dsum(S, p):
                    # blocks (2p, 2p+1): scatter per-half sums into d rows
                    if S not in d_ps:
                        d_ps[S] = ps_d.tile([128, 512], F32, tag="d", name="d_ps")
                    for w in range(2):
                        i = 2 * p + w
                        nc.tensor.matmul(
                            d_ps[S],
                            lhsT=g64_sb[:, 62 - 2 * i:190 - 2 * i],
                            rhs=e_tiles[(S, i)],
                            start=(i == 0),
                            stop=(i == NI - 1),
                        )

                def emit_recip(S):
                    rd32 = rdp.tile([128, 512], F32, tag="rd32", name="rd32")
                    nc.vector.reciprocal_approx_fast(out=rd32, in_=d_ps.pop(S))
                    rd = rdp.tile([128, 512], BF16, tag="rd", name="rd")
                    nc.scalar.copy(rd, rd32)
                    rd_sb[S] = rd

                def emit_final(S):
                    # A@V already carries the Wcr projection (folded into wvt
                    # on the host): just add the combined bias and ship out
                    ah = att_h.pop(S)
                    out_t = obufp.tile([128, 2, 512], BF16, tag="out_t", name="out_t")
                    for g in range(2):
                        if g == 0:
                            nc.scalar.add(out_t[:, g, :], ah[g], bfin_sb[:, g:g + 1])
                        else:
                            nc.vector.tensor_scalar_add(out_t[:, g, :], ah[g], bfin_sb[:, g:g + 1])
                        nc.sync.dma_start(
                            out=out_d[g * 128:(g + 1) * 128, S * 512:(S + 1) * 512],
                            in_=out_t[:, g, :],
                        )

                es_live = {}   # (S, t) -> es pair tile from emit_bc2

                def emit_v(p):
                    # V^T for blocks (2p, 2p+1), interleaved into S0 (ScalarE-
                    # bound there, so these PE/DVE ops ride along free).
                    # Borrows the bc PSUM banks, idle until S1.
                    psv = ps_bc.tile([128, 512], F32, tag=f"bc{p % 2}", name="psv")
                    for bb in range(2):
                        i = 2 * p + bb
                        for k in range(4):
                            nc.tensor.matmul(
                                psv[:, bb * 256:(bb + 1) * 256],
                                lhsT=x_sb[k][:, i * 128:(i + 1) * 128],
                                rhs=wvt_sb[:, k, :],
                                start=(k == 0),
                                stop=(k == 3),
                            )
                    nc.vector.tensor_copy(vt_sb[:, 2 * p:2 * p + 2, :], psv)

                def emit_bc2(S, t, drain=False):
                    # pair t -> blocks (2t, 2t+1): row-packed 1/d broadcast
                    pair = []
                    for w in range(2):
                        ps = ps_bc.tile([128, 512], F32, tag=f"bc{w}", name=f"bc{w}")
                        nc.tensor.matmul(
                            ps,
                            lhsT=g128_sb[64 * w:64 * w + 64, t, :],
                            rhs=rd_sb[S][64 * w:64 * w + 64, :],
                            start=True,
                            stop=True,
                        )
                        es = esp.tile([128, 512], BF16, tag=f"es{w}", name=f"es{w}")
                        with nc.allow_low_precision(reason="attn weights are bf16 by design"):
                            if drain and w == 1:
                                # drain phase is DVE-paced: stage the scale in
                                # SBUF via the idle ScalarE so the multiply
                                # runs at the 2x 16-bit SBUF rate
                                sc_sb = esp.tile([128, 512], BF16, tag="scsb", name="sc_sb")
                                nc.scalar.copy(sc_sb, ps)
                                nc.vector.tensor_mul(es, e_tiles[(S, 2 * t + w)], sc_sb)
                            else:
                                nc.vector.tensor_mul(es, e_tiles[(S, 2 * t + w)], ps)
                        pair.append(es)
                    es_live[(S, t)] = pair

                def emit_av2(S, t):
                    if S not in att_h:
                        att_h[S] = [
                            ps_att.tile([128, 512], F32, tag=f"att{h}", name=f"att{h}")
                            for h in range(2)
                        ]
                    pair = es_live.pop((S, t))
                    for w in range(2):
                        blk = 2 * t + w
                        for h in range(2):
                            nc.tensor.matmul(
                                att_h[S][h],
                                lhsT=vt_sb[:, blk, h * 128:(h + 1) * 128],
                                rhs=pair[w],
                                start=(t == 0 and w == 0),
                                stop=(t == NP - 1 and w == 1),
                            )

                # Software pipeline: consumer side (bc/av/final of S-1) trails
                # the producer side (scores/dsum of S) far enough that the
                # reciprocal latency at each super-block boundary is covered.
                for S in range(NSUP + 1):
                    prod = S < NSUP
                    cons = S >= 1
                    drain = not prod
                    for p in range(NP):
                        if prod:
                            emit_scores(S, p)
                        # bc pair right after scores pair: the tiled (half-
                        # array) matmuls cluster so fewer LDW serialization
                        # boundaries are paid per iteration
                        if cons and p >= 1:
                            emit_bc2(S - 1, p - 1, drain)
                        if prod and p >= 1:
                            emit_dsum(S, p - 1)
                        if S == 0:
                            if 2 <= p < 6:
                                # remaining QK chunks ride along S0 (borrow
                                # the idle att PSUM banks)
                                emit_qk(2 + p, ps_att, f"att{p % 2}")
                                if p == 5:
                                    nc.sync.dma_start(out=k2_sb[0:64, 8:16, :], in_=qkb[64:128, 4:8, :, 0, :])
                                    nc.sync.dma_start(out=k2_sb[64:128, 8:16, :], in_=qkb[64:128, 4:8, :, 1, :])
                            if p >= 8:
                                emit_v(p)
                        if cons and p >= 2:
                            emit_av2(S - 1, p - 2)
                    if prod:
                        emit_dsum(S, NP - 1)
                        emit_recip(S)
                    if cons:
                        emit_bc2(S - 1, NP - 1, drain)
                        emit_av2(S - 1, NP - 2)
                        emit_av2(S - 1, NP - 1)
                        emit_final(S - 1)
    nc.compile()
    return nc


def get_nc():
    if "nc" not in _CACHE:
        _CACHE["nc"] = _build_nc()
    return _CACHE["nc"]


def make_in_maps(inputs):
    rgb = np.asarray(inputs["rgb_features"], np.float32)
    chm = np.asarray(inputs["chm_features"], np.float32)
    Wq = np.asarray(inputs["Wq"], np.float32)
    bq = np.asarray(inputs["bq"], np.float32)
    Wk = np.asarray(inputs["Wk"], np.float32)
    bk = np.asarray(inputs["bk"], np.float32)
    Wv = np.asarray(inputs["Wv"], np.float32)
    bv = np.asarray(inputs["bv"], np.float32)
    Wcr = np.asarray(inputs["Wcr"], np.float32)
    bcr = np.asarray(inputs["bcr"], np.float32)

    Wqk = np.concatenate([Wq, Wk], axis=0)  # [128, 512]
    wqk = np.ascontiguousarray(Wqk.T.reshape(4, 128, 128).transpose(1, 0, 2)).astype(ml_dtypes.bfloat16)
    # fold the output projection into V: Wcr (A V^T)^T == A (Wcv x)^T with
    # Wcv = Wcr @ Wv, and the biases collapse to 64*Wcr@bv + bcr because each
    # query's attention weights sum to 64 (one softmax per x-group, 64 groups)
    Wcv = Wcr @ Wv
    wvt = np.ascontiguousarray(Wcv.T.reshape(4, 128, 256).transpose(1, 0, 2)).astype(ml_dtypes.bfloat16)
    bqk = np.ascontiguousarray(np.concatenate([bq, bk]).reshape(128, 1))
    bfin = np.ascontiguousarray((64.0 * (Wcr @ bv) + bcr).reshape(2, 128).T)

    # dsum selector, shift-base form: slice for block i is g64b[:, 62-2i:190-2i]
    # with ones at u = 62+p//64 (d rows 2i+p//64) and u = 126+p//64 (dup rows
    # 64+2i+p//64, keeping every d_ps row finite for the reciprocal).
    p_ix = np.arange(128)
    g64b = np.zeros((128, 190), ml_dtypes.bfloat16)
    g64b[p_ix, 62 + p_ix // 64] = 1
    g64b[p_ix, 126 + p_ix // 64] = 1

    # row-packed 1/d broadcast selectors: pair t -> blocks (2t, 2t+1).
    # window0 (partitions 0:64) serves block 2t from rd rows 4t + m//64;
    # window1 (partitions 64:128) serves block 2t+1 from the dup rows
    # (rd[64+j] = 1/d[j]).
    g128p = np.zeros((128, 16, 128), ml_dtypes.bfloat16)
    m_ix = np.arange(128)
    for t in range(16):
        g128p[4 * t + m_ix // 64, t, m_ix] = 1
        g128p[64 + 4 * t + 2 + m_ix // 64, t, m_ix] = 1

    in_maps = []
    for core in range(8):
        b, par = divmod(core, 2)
        x = np.concatenate([rgb[b], chm[b]], axis=0).reshape(CIN, HW)
        if par:
            x = np.roll(x, -QCOLS, axis=1)
        x = np.ascontiguousarray(x)
        in_maps.append(
            {
                "x": x.astype(ml_dtypes.bfloat16),
                "wqk": wqk,
                "wvt": wvt,
                "bqk": bqk,
                "bfin": bfin,
                "g64b": g64b,
                "g128p": g128p,
            }
        )
    return in_maps


def assemble(outs):
    full = np.empty((B, C, HW), np.float32)
    for core in range(8):
        b, par = divmod(core, 2)
        full[b, :, par * QCOLS:(par + 1) * QCOLS] = np.asarray(outs[core], np.float32)
    return full.reshape(B, C, H, W)


def kernel(**inputs):
    from concourse.bass_utils import run_bass_kernel_spmd

    nc = get_nc()
    res = run_bass_kernel_spmd(nc, make_in_maps(inputs), core_ids=list(range(8)))
    return assemble([r["out"] for r in res.results])


# revision 46
# speedup vs baseline: 1.0259x; 1.0259x over previous
"""Cross-attention-concat kernel for Trainium2 (8 NeuronCores, Bass/Tile).

Math (per batch b):
  x   = concat(rgb, chm) on channels           [512, 4096]   (pixels hw = h*64+w)
  Q   = Wq x + bq ; K = Wk x + bk              [64, ...]
  V   = Wv x + bv                              [256, 4096]
  S   = Q^T K                                  [2048 hw, 4096 xy]
  A   = softmax over y within each x-group of 64 keys
  out = Wcr (A V^T)^T + bcr                    [256, 2048]

Sharding: core = (batch, H-half). The host rolls each batch's pixel axis by
2048*(core%2) so every core runs the same program with its queries at
columns 0:2048 of the rolled image.

Key structure (v2):
- Transposed-scores formulation: S^T [xy, q] per 128-xy block, exp on
  ScalarE, per-x-group softmax denominators collected by a selector matmul
  (g64), reciprocal broadcast back by a second selector matmul (g128),
  scale fused into the PSUM->SBUF pass feeding A@V.
- The two K=64-contraction matmuls (scores: c=64; 1/d broadcast: rd rows)
  are ROW-PACKED: two concurrent matmuls on PE row-halves via
  tile_position, halving their stream time. K blocks are packed
  even/odd into partition halves (k2), Q is duplicated into both halves,
  rd carries d duplicated into both halves (via g64's dup columns).
- Preamble is DMA-pipelined column-chunk-wise with a fused [Wq|Wk]
  stationary; Q/K land via small SBUF->SBUF shuffle DMAs.
- Reciprocal uses the fast approx DVE op and is emitted one pipeline
  stage early so it never stalls the PE.
"""

import numpy as np
import ml_dtypes

B, C, H, W = 4, 256, 64, 64
HW = H * W               # 4096
CIN = 2 * C              # 512
QCOLS = HW // 2          # 2048 queries per core
NSUP = QCOLS // 512      # 4 super-blocks of 512 queries
NI = HW // 128           # 32 xy-blocks of 128
NP = NI // 2             # 16 block-pairs

_CACHE = {}


def _build_nc():
    import concourse.bacc as bacc
    import concourse.tile as tile
    from concourse import mybir

    F32 = mybir.dt.float32
    BF16 = mybir.dt.bfloat16
    AF = mybir.ActivationFunctionType

    nc = bacc.Bacc("TRN2", target_bir_lowering=False, debug=False, num_devices=8)

    x_d = nc.dram_tensor("x", [CIN, HW], BF16, kind="ExternalInput").ap()
    g64_d = nc.dram_tensor("g64b", [128, 190], BF16, kind="ExternalInput").ap()
    g128_d = nc.dram_tensor("g128p", [128, 16, 128], BF16, kind="ExternalInput").ap()
    wqk_d = nc.dram_tensor("wqk", [128, 4, 128], BF16, kind="ExternalInput").ap()
    # wvt carries (Wcr @ Wv)^T: the output projection is folded into V on the
    # host, so the A@V matmul directly produces the projected output
    wvt_d = nc.dram_tensor("wvt", [128, 4, 256], BF16, kind="ExternalInput").ap()
    bqk_d = nc.dram_tensor("bqk", [128, 1], F32, kind="ExternalInput").ap()
    bfin_d = nc.dram_tensor("bfin", [128, 2], F32, kind="ExternalInput").ap()
    out_d = nc.dram_tensor("out", [C, QCOLS], BF16, kind="ExternalOutput").ap()

    with tile.TileContext(nc) as tc:
        with (
            tc.tile_pool(name="const", bufs=1) as constp,
            tc.tile_pool(name="qkv", bufs=1) as qkvp,
            tc.tile_pool(name="pbuf", bufs=40) as pbufp,
            tc.tile_pool(name="esbuf", bufs=2) as esp,
            tc.tile_pool(name="rdbuf", bufs=2) as rdp,
            tc.tile_pool(name="obuf", bufs=2) as obufp,
        ):
            # ---- constants (weights first; big selector consts after x) ----
            wqk_sb = constp.tile([128, 4, 128], BF16)
            bqk_sb = constp.tile([128, 1], F32)
            nc.sync.dma_start(out=wqk_sb, in_=wqk_d)
            nc.sync.dma_start(out=bqk_sb, in_=bqk_d)

            q_sb = qkvp.tile([128, QCOLS], BF16)       # Q dup'd in both halves
            k2_sb = qkvp.tile([128, NP, 128], BF16)    # K blocks even/odd packed
            vt_sb = qkvp.tile([128, NI, 256], BF16)    # V^T [xy-block, 128, 256]

            wvt_sb = constp.tile([128, 4, 256], BF16)
            bfin_sb = constp.tile([128, 2], F32)
            g64_sb = constp.tile([128, 190], BF16)
            g128_sb = constp.tile([128, 16, 128], BF16)

            # ---- preamble: 2 big x chunks, warm-up MMs during the DMA head,
            # QK projection only (V is interleaved into S0 below) ----
            x_sb = [qkvp.tile([128, HW], BF16, tag=f"x{k}", name=f"x{k}") for k in range(4)]
            # qkb[:, j, a, b, :] : col 512j+256a+128b; rows 0:64 = Q, 64:128 = K
            qkb = qkvp.tile([128, 8, 2, 2, 128], BF16, name="qkb")

            def emit_v2(p, pool, tag, bufs=1):
                # V^T for blocks (2p, 2p+1)
                psv = pool.tile([128, 512], F32, tag=tag, bufs=bufs, name="psv")
                for bb in range(2):
                    i = 2 * p + bb
                    for k in range(4):
                        nc.tensor.matmul(
                            psv[:, bb * 256:(bb + 1) * 256],
                            lhsT=x_sb[k][:, i * 128:(i + 1) * 128],
                            rhs=wvt_sb[:, k, :],
                            start=(k == 0),
                            stop=(k == 3),
                        )
                nc.vector.tensor_copy(vt_sb[:, 2 * p:2 * p + 2, :], psv)

            def emit_qk(j, pool, tag, bufs=1):
                sl = slice(j * 512, (j + 1) * 512)
                psqk = pool.tile([128, 512], F32, tag=tag, bufs=bufs, name="psqk")
                for k in range(4):
                    nc.tensor.matmul(
                        psqk,
                        lhsT=wqk_sb[:, k, :],
                        rhs=x_sb[k][:, sl],
                        start=(k == 0),
                        stop=(k == 3),
                    )
                if j % 2 == 0:
                    nc.scalar.add(qkb[:, j], psqk, bqk_sb)
                else:
                    nc.vector.tensor_scalar_add(qkb[:, j], psqk, bqk_sb)
            with tc.tile_pool(name="ps_pre", bufs=1, space="PSUM") as ps_pre:
                # warm-up MMs: trip the PE HAM throttle to full clock and keep
                # it busy while the x chunks stream in. The operand comes from
                # an on-device iota so warm-up starts before any DMA lands.
                warm_sb = qkvp.tile([128, 512], BF16, name="warm_sb")
                nc.gpsimd.iota(warm_sb, [[1, 512]], base=0, channel_multiplier=0,
                               allow_small_or_imprecise_dtypes=True)
                junk = ps_pre.tile([128, 512], F32, tag="junk", name="junk")
                for _ in range(26):
                    nc.tensor.matmul(junk, lhsT=warm_sb[:, 0:128], rhs=warm_sb, start=True, stop=True)
                # x descriptors split across the two HWDGE engines (Sync +
                # Scalar) so descriptor generation isn't serialized; first
                # chunk small so the QK matmuls can start early. wvt leads the
                # scalar queue — the preamble V pairs need it right away.
                nc.scalar.dma_start(out=wvt_sb, in_=wvt_d)
                for lo, hi in ((0, 1024), (1024, 2048), (2048, 4096)):
                    for k in range(4):
                        eng = nc.sync if k < 2 else nc.scalar
                        eng.dma_start(out=x_sb[k][:, lo:hi], in_=x_d[k * 128:(k + 1) * 128, lo:hi])
                nc.sync.dma_start(out=g64_sb, in_=g64_d)
                nc.sync.dma_start(out=bfin_sb, in_=bfin_d)

                # interleave early V pairs as PE filler between QK chunks —
                # they only need x columns that have already landed, covering
                # the DMA wait for the next x chunk
                emit_qk(0, ps_pre, "psqk", 2)
                emit_qk(1, ps_pre, "psqk", 2)
                for p in range(4):
                    emit_v2(p, ps_pre, "psv", 2)
                emit_qk(2, ps_pre, "psqk", 2)
                emit_qk(3, ps_pre, "psqk", 2)
                for p in range(4, 8):
                    emit_v2(p, ps_pre, "psv", 2)
                # Q dup + K shuffle halves on the scalar queue: issued ahead
                # of the sync queue's x backlog, so S0's start is gated by
                # data readiness rather than descriptor-issue serialization.
                nc.scalar.dma_start(out=q_sb[0:64, :], in_=qkb[0:64, 0:4])
                nc.scalar.dma_start(out=q_sb[64:128, :], in_=qkb[0:64, 0:4])
                # even xy-blocks -> window0, odd -> window1
                nc.scalar.dma_start(out=k2_sb[0:64, 0:8, :], in_=qkb[64:128, 0:4, :, 0, :])
                nc.scalar.dma_start(out=k2_sb[64:128, 0:8, :], in_=qkb[64:128, 0:4, :, 1, :])
                # g128p is not needed until S1 — issue it after the shuffles
                nc.scalar.dma_start(out=g128_sb, in_=g128_d)

            # ---- main loop ----
            with (
                tc.tile_pool(name="ps_sc", bufs=1, space="PSUM") as ps_sc,
                tc.tile_pool(name="ps_d", bufs=1, space="PSUM") as ps_d,
                tc.tile_pool(name="ps_bc", bufs=1, space="PSUM") as ps_bc,
                tc.tile_pool(name="ps_att", bufs=1, space="PSUM") as ps_att,
            ):
                e_tiles = {}   # (S, p) -> exp pair tile, bf16 [128, 2, 512]
                d_ps = {}      # S -> PSUM [128, 512] per-x-group sums (dup'd halves)
                rd_sb = {}     # S -> SBUF bf16 [128, 512] reciprocal sums
                att_h = {}     # S -> accumulating PSUM pair

                def emit_scores(S, p):
                    # pair p -> xy-blocks (2p, 2p+1) on PE row-halves.
                    # Separate single-buffered banks + per-block exps keep the
                    # PE->ScalarE->PE reuse chain shorter than the PE work per
                    # iteration, so the PE never waits.
                    q0 = S * 512
                    for w in range(2):
                        ps = ps_sc.tile([128, 512], F32, tag=f"sc{w}", name=f"sc{w}")
                        nc.tensor.matmul(
                            ps,
                            lhsT=k2_sb[64 * w:64 * w + 64, p, :],
                            rhs=q_sb[64 * w:64 * w + 64, q0:q0 + 512],
                            start=True,
                            stop=True,
                        )
                        e = pbufp.tile([128, 512], BF16, tag="e", name="e")
                        nc.scalar.activation(e, ps, AF.Exp)
                        e_tiles[(S, 2 * p + w)] = e

                def emit_dsum(S, p):
                    # blocks (2p, 2p+1): scatter per-half sums into d rows
                    if S not in d_ps:
                        d_ps[S] = ps_d.tile([128, 512], F32, tag="d", name="d_ps")
                    for w in range(2):
                        i = 2 * p + w
                        nc.tensor.matmul(
                            d_ps[S],
                            lhsT=g64_sb[:, 62 - 2 * i:190 - 2 * i],
                            rhs=e_tiles[(S, i)],
                            start=(i == 0),
                            stop=(i == NI - 1),
                        )

                def emit_recip(S):
                    rd32 = rdp.tile([128, 512], F32, tag="rd32", name="rd32")
                    nc.vector.reciprocal_approx_fast(out=rd32, in_=d_ps.pop(S))
                    rd = rdp.tile([128, 512], BF16, tag="rd", name="rd")
                    nc.scalar.copy(rd, rd32)
                    rd_sb[S] = rd

                def emit_final(S):
                    # A@V already carries the Wcr projection (folded into wvt
                    # on the host): just add the combined bias and ship out
                    ah = att_h.pop(S)
                    out_t = obufp.tile([128, 2, 512], BF16, tag="out_t", name="out_t")
                    for g in range(2):
                        if g == 0:
                            nc.scalar.add(out_t[:, g, :], ah[g], bfin_sb[:, g:g + 1])
                        else:
                            nc.vector.tensor_scalar_add(out_t[:, g, :], ah[g], bfin_sb[:, g:g + 1])
                        nc.sync.dma_start(
                            out=out_d[g * 128:(g + 1) * 128, S * 512:(S + 1) * 512],
                            in_=out_t[:, g, :],
                        )

                es_live = {}   # (S, t) -> es pair tile from emit_bc2

                def emit_v(p):
                    # V^T for blocks (2p, 2p+1), interleaved into S0 (ScalarE-
                    # bound there, so these PE/DVE ops ride along free).
                    # Borrows the bc PSUM banks, idle until S1.
                    psv = ps_bc.tile([128, 512], F32, tag=f"bc{p % 2}", name="psv")
                    for bb in range(2):
                        i = 2 * p + bb
                        for k in range(4):
                            nc.tensor.matmul(
                                psv[:, bb * 256:(bb + 1) * 256],
                                lhsT=x_sb[k][:, i * 128:(i + 1) * 128],
                                rhs=wvt_sb[:, k, :],
                                start=(k == 0),
                                stop=(k == 3),
                            )
                    nc.vector.tensor_copy(vt_sb[:, 2 * p:2 * p + 2, :], psv)

                def emit_bc2(S, t, drain=False):
                    # pair t -> blocks (2t, 2t+1): row-packed 1/d broadcast
                    pair = []
                    for w in range(2):
                        ps = ps_bc.tile([128, 512], F32, tag=f"bc{w}", name=f"bc{w}")
                        nc.tensor.matmul(
                            ps,
                            lhsT=g128_sb[64 * w:64 * w + 64, t, :],
                            rhs=rd_sb[S][64 * w:64 * w + 64, :],
                            start=True,
                            stop=True,
                        )
                        es = esp.tile([128, 512], BF16, tag=f"es{w}", name=f"es{w}")
                        with nc.allow_low_precision(reason="attn weights are bf16 by design"):
                            if drain and w == 1:
                                # drain phase is DVE-paced: stage the scale in
                                # SBUF via the idle ScalarE so the multiply
                                # runs at the 2x 16-bit SBUF rate
                                sc_sb = esp.tile([128, 512], BF16, tag="scsb", name="sc_sb")
                                nc.scalar.copy(sc_sb, ps)
                                nc.vector.tensor_mul(es, e_tiles[(S, 2 * t + w)], sc_sb)
                            else:
                                nc.vector.tensor_mul(es, e_tiles[(S, 2 * t + w)], ps)
                        pair.append(es)
                    es_live[(S, t)] = pair

                def emit_av2(S, t):
                    if S not in att_h:
                        att_h[S] = [
                            ps_att.tile([128, 512], F32, tag=f"att{h}", name=f"att{h}")
                            for h in range(2)
                        ]
                    pair = es_live.pop((S, t))
                    for w in range(2):
                        blk = 2 * t + w
                        for h in range(2):
                            nc.tensor.matmul(
                                att_h[S][h],
                                lhsT=vt_sb[:, blk, h * 128:(h + 1) * 128],
                                rhs=pair[w],
                                start=(t == 0 and w == 0),
                                stop=(t == NP - 1 and w == 1),
                            )

                # Software pipeline: consumer side (bc/av/final of S-1) trails
                # the producer side (scores/dsum of S) far enough that the
                # reciprocal latency at each super-block boundary is covered.
                for S in range(NSUP + 1):
                    prod = S < NSUP
                    cons = S >= 1
                    drain = not prod
                    for p in range(NP):
                        if prod:
                            emit_scores(S, p)
                        # bc pair right after scores pair: the tiled (half-
                        # array) matmuls cluster so fewer LDW serialization
                        # boundaries are paid per iteration
                        if cons and p >= 1:
                            emit_bc2(S - 1, p - 1, drain)
                        if prod and p >= 1:
                            emit_dsum(S, p - 1)
                        if S == 0:
                            if 2 <= p < 6:
                                # remaining QK chunks ride along S0 (borrow
                                # the idle att PSUM banks)
                                emit_qk(2 + p, ps_att, f"att{p % 2}")
                                if p == 5:
                                    nc.scalar.dma_start(out=k2_sb[0:64, 8:16, :], in_=qkb[64:128, 4:8, :, 0, :])
                                    nc.scalar.dma_start(out=k2_sb[64:128, 8:16, :], in_=qkb[64:128, 4:8, :, 1, :])
                            if p >= 8:
                                emit_v(p)
                        if cons and p >= 2:
                            emit_av2(S - 1, p - 2)
                    if prod:
                        emit_dsum(S, NP - 1)
                        emit_recip(S)
                    if cons:
                        emit_bc2(S - 1, NP - 1, drain)
                        emit_av2(S - 1, NP - 2)
                        emit_av2(S - 1, NP - 1)
                        emit_final(S - 1)
    nc.compile()
    return nc


def get_nc():
    if "nc" not in _CACHE:
        _CACHE["nc"] = _build_nc()
    return _CACHE["nc"]


def make_in_maps(inputs):
    rgb = np.asarray(inputs["rgb_features"], np.float32)
    chm = np.asarray(inputs["chm_features"], np.float32)
    Wq = np.asarray(inputs["Wq"], np.float32)
    bq = np.asarray(inputs["bq"], np.float32)
    Wk = np.asarray(inputs["Wk"], np.float32)
    bk = np.asarray(inputs["bk"], np.float32)
    Wv = np.asarray(inputs["Wv"], np.float32)
    bv = np.asarray(inputs["bv"], np.float32)
    Wcr = np.asarray(inputs["Wcr"], np.float32)
    bcr = np.asarray(inputs["bcr"], np.float32)

    Wqk = np.concatenate([Wq, Wk], axis=0)  # [128, 512]
    wqk = np.ascontiguousarray(Wqk.T.reshape(4, 128, 128).transpose(1, 0, 2)).astype(ml_dtypes.bfloat16)
    # fold the output projection into V: Wcr (A V^T)^T == A (Wcv x)^T with
    # Wcv = Wcr @ Wv, and the biases collapse to 64*Wcr@bv + bcr because each
    # query's attention weights sum to 64 (one softmax per x-group, 64 groups)
    Wcv = Wcr @ Wv
    wvt = np.ascontiguousarray(Wcv.T.reshape(4, 128, 256).transpose(1, 0, 2)).astype(ml_dtypes.bfloat16)
    bqk = np.ascontiguousarray(np.concatenate([bq, bk]).reshape(128, 1))
    bfin = np.ascontiguousarray((64.0 * (Wcr @ bv) + bcr).reshape(2, 128).T)

    # dsum selector, shift-base form: slice for block i is g64b[:, 62-2i:190-2i]
    # with ones at u = 62+p//64 (d rows 2i+p//64) and u = 126+p//64 (dup rows
    # 64+2i+p//64, keeping every d_ps row finite for the reciprocal).
    p_ix = np.arange(128)
    g64b = np.zeros((128, 190), ml_dtypes.bfloat16)
    g64b[p_ix, 62 + p_ix // 64] = 1
    g64b[p_ix, 126 + p_ix // 64] = 1

    # row-packed 1/d broadcast selectors: pair t -> blocks (2t, 2t+1).
    # window0 (partitions 0:64) serves block 2t from rd rows 4t + m//64;
    # window1 (partitions 64:128) serves block 2t+1 from the dup rows
    # (rd[64+j] = 1/d[j]).
    g128p = np.zeros((128, 16, 128), ml_dtypes.bfloat16)
    m_ix = np.arange(128)
    for t in range(16):
        g128p[4 * t + m_ix // 64, t, m_ix] = 1
        g128p[64 + 4 * t + 2 + m_ix // 64, t, m_ix] = 1

    in_maps = []
    for core in range(8):
        b, par = divmod(core, 2)
        x = np.concatenate([rgb[b], chm[b]], axis=0).reshape(CIN, HW)
        if par:
            x = np.roll(x, -QCOLS, axis=1)
        x = np.ascontiguousarray(x)
        in_maps.append(
            {
                "x": x.astype(ml_dtypes.bfloat16),
                "wqk": wqk,
                "wvt": wvt,
                "bqk": bqk,
                "bfin": bfin,
                "g64b": g64b,
                "g128p": g128p,
            }
        )
    return in_maps


def assemble(outs):
    full = np.empty((B, C, HW), np.float32)
    for core in range(8):
        b, par = divmod(core, 2)
        full[b, :, par * QCOLS:(par + 1) * QCOLS] = np.asarray(outs[core], np.float32)
    return full.reshape(B, C, H, W)


def kernel(**inputs):
    from concourse.bass_utils import run_bass_kernel_spmd

    nc = get_nc()
    res = run_bass_kernel_spmd(nc, make_in_maps(inputs), core_ids=list(range(8)))
    return assemble([r["out"] for r in res.results])
